# revision 24
# baseline (speedup 1.0000x reference)
"""Trainium2 Bass kernel for nn_ContrastiveLoss (SimCLR-style, N=8192, D=128).

v4: symmetry + host layout prep.  Global rows form 64 blocks of 128; core c
owns blocks 8c..8c+7.  The host normalizes z, rotates it per core, and ships
the TRANSPOSED bf16 matrix znT [128, 5120] (local col-blocks 0..39), so the
device does zero layout work.  Per local row-block b (strip), the device
computes sim blocks at offsets d=0..32 only (half the matrix, wrap-band):
three chunks of 1536/1536/1152 columns, each matmul'd into a rotating PSUM
tile and exponentiated by ACT (exp(10x-10), bf16 out, fp32 accum_out gives
the row sums).  Transposed contributions (offsets 33..63 of each row) come
from column sums of the d=1..31 blocks: ones[128,32]^T @ E matmuls accumulate
into a packed CS PSUM region (seg s of 512 abs cols -> partition-group slot
via tile_position; bank 6 is drained mid-kernel and recycled for segs 8-9).
Colsums of chunk k are emitted after the fills of chunk k+1 so the in-order
PE queue never stalls on ACT.  Host: scatter colsums to owning rows, add row
sums, subtract diag exp(0)=1, then loss = mean(ln S + 10 - 10*cos_pos).
"""

import sys

sys.path.insert(0, "/opt/trn_rl_repo")

from contextlib import ExitStack

import numpy as np

import concourse.bass as bass
import concourse.bacc as bacc
import concourse.tile as tile
from concourse import mybir
from concourse import bass_utils

B = 4096
D = 128
N = 2 * B
NCORES = 8
ROWS = N // NCORES   # 1024 rows per core
NBLK = 8             # strips per core
NT = 40              # znT col-blocks per core (local blocks 0..39)
ZCOLS = NT * 128     # 5120
SEG = 512
CHUNK = 1536         # chunk 0/1 width; chunk 2 is 1152
INV_T = 10.0
EPS = 1e-8

F32 = mybir.dt.float32
BF16 = mybir.dt.bfloat16
AX = mybir.AxisListType
AF = mybir.ActivationFunctionType


def _cs_slot(seg, life2=False):
    """Colsum psum placement: seg (512 abs cols) -> (bank_col, grp)."""
    if life2:  # segs 8, 9 reuse bank col 0 after the mid-kernel drain
        return 0, seg - 8
    if seg < 4:
        return 0, seg
    return 512, seg - 4


def _chunk_w(dc):
    return CHUNK if dc < 2 else 1152


def _build() -> bass.Bass:
    nc = bacc.Bacc(None)
    zT_in = nc.declare_dram_parameter("zT", [128, ZCOLS], BF16, isOutput=False)
    out_acc = nc.declare_dram_parameter("acc", [128, 3 * NBLK], F32, isOutput=True)
    out_cs = nc.declare_dram_parameter("cs", [4, 1536], F32, isOutput=True)

    with tile.TileContext(nc) as tc:
        with ExitStack() as ctx:
            persist = ctx.enter_context(tc.tile_pool(name="persist", bufs=1))
            epool = ctx.enter_context(tc.tile_pool(name="epool", bufs=3))
            ptpool = ctx.enter_context(tc.tile_pool(name="ptpool", bufs=2, space="PSUM"))
            psB = ctx.enter_context(tc.tile_pool(name="psB", bufs=1, space="PSUM"))

            warm_src = persist.tile([128, 512], BF16)
            nc.vector.memset(warm_src, 1.0)
            ones32 = persist.tile([128, 32], BF16)
            nc.vector.memset(ones32, 1.0)
            b_neg10 = persist.tile([128, 1], F32)
            nc.vector.memset(b_neg10, -INV_T)

            znT = persist.tile([128, ZCOLS], BF16)
            acc_sb = persist.tile([128, 3 * NBLK], F32)
            cs_sb = persist.tile([128, 1536], F32)

            CS = psB.tile([128, 1024], F32)       # 2 banks
            nc.vector.memset(CS, 0.0)

            # input DMA in pieces so compute can start early
            for p0, p1 in ((0, 1664), (1664, 3200), (3200, 4352), (4352, ZCOLS)):
                nc.sync.dma_start(out=znT[:, p0:p1], in_=zT_in[:, p0:p1])

            # PE warmup during the DMA wait: ~4us of dummy matmuls flips the
            # HAM clock gate to 2.4GHz before the real fills arrive; results
            # land in a psum slot later overwritten by the real fills.
            wpt = ptpool.tile([128, CHUNK], F32, tag="pt", name="warm")
            for _ in range(10):
                nc.tensor.matmul(
                    wpt[0:32, 0:512], warm_src[:, 0:32], warm_src, start=True, stop=True,
                )

            cs_life2 = {"on": False}

            # static last-writer per CS bank-life for stop flags
            chunk_order = [(dc, b) for dc in range(3) for b in range(NBLK)]
            last_of = {}
            for (dc, b) in chunk_order:
                for d in range(max(1, 12 * dc), min(32, 12 * dc + 12)):
                    jb = b + d
                    seg = jb // 4
                    bank = _cs_slot(seg, seg >= 8)[0]
                    last_of[(bank, seg >= 8)] = (dc, b, d)

            def colsum_mms(dc, b, E):
                d = max(1, 12 * dc)
                d_hi = min(32, 12 * dc + 12)
                while d < d_hi:
                    jb = b + d
                    seg = jb // 4
                    life2 = seg >= 8
                    if life2 and not cs_life2["on"]:
                        raise RuntimeError("life2 before drain")
                    bank, grp = _cs_slot(seg, life2)
                    run = 1
                    while (
                        d + run < d_hi
                        and (b + d + run) // 4 == seg
                    ):
                        run += 1
                    stop = any(
                        last_of.get((bank, life2)) == (dc, b, dd)
                        for dd in range(d, d + run)
                    )
                    k = d - 12 * dc
                    off = (jb * 128) % 512
                    nc.tensor.matmul(
                        CS[32 * grp : 32 * grp + 32, bank + off : bank + off + run * 128],
                        ones32,
                        E[:, k * 128 : (k + run) * 128],
                        start=False,
                        stop=stop,
                        tile_position=(0, 32 * grp),
                        skip_group_check=True,
                    )
                    d += run

            pend = None
            for (dc, b) in chunk_order:
                W = _chunk_w(dc)
                pt = ptpool.tile([128, CHUNK], F32, tag="pt", name="pt")
                c0 = b * 128 + dc * CHUNK
                off = 0
                while off < W:
                    w = min(SEG, W - off)
                    nc.tensor.matmul(
                        pt[:, off : off + w],
                        znT[:, b * 128 : (b + 1) * 128],
                        znT[:, c0 + off : c0 + off + w],
                        start=True,
                        stop=True,
                    )
                    off += w
                if pend is not None:
                    if pend[0][:2] == (2, 1):
                        # drain bank 6 (segs 0..3), recycle for segs 8..9;
                        # colsums of chunk (2,1) write seg 8 first
                        nc.vector.tensor_copy(cs_sb[:, 0:512], CS[:, 0:512])
                        nc.vector.memset(CS[:, 0:512], 0.0)
                        cs_life2["on"] = True
                    colsum_mms(*pend[0][:2], pend[1])
                E = epool.tile([128, CHUNK], BF16, tag="E", name="E")
                acc_slot = acc_sb[:, 3 * b + dc : 3 * b + dc + 1]
                if dc < 2:
                    nc.scalar.activation(
                        E[:, 0:W], pt[:, 0:W], AF.Exp, scale=INV_T, bias=b_neg10,
                        accum_out=acc_slot,
                    )
                else:
                    # short chunks: row-sum on the idle DVE instead of paying
                    # ACT's read-accumulator tax
                    nc.scalar.activation(
                        E[:, 0:W], pt[:, 0:W], AF.Exp, scale=INV_T, bias=b_neg10,
                    )
                    nc.vector.reduce_sum(acc_slot, E[:, 0:W], axis=AX.X)
                pend = ((dc, b), E)
            if pend[0][:2] == (2, 1):
                raise RuntimeError("unexpected")
            colsum_mms(*pend[0][:2], pend[1])

            nc.vector.tensor_copy(cs_sb[:, 512:1536], CS[:, :])
            nc.sync.dma_start(out=out_acc[:, :], in_=acc_sb)
            # only partition rows 0/32/64/96 of cs_sb carry data
            nc.sync.dma_start(out=out_cs[:, :], in_=cs_sb[0:97:32, :])

    nc.compile()
    return nc


_NC = None


def _get_nc() -> bass.Bass:
    global _NC
    if _NC is None:
        _NC = _build()
    return _NC


def prepare_in_maps(emb0: np.ndarray, emb1: np.ndarray):
    import ml_dtypes

    z = np.concatenate(
        [np.asarray(emb0, np.float32), np.asarray(emb1, np.float32)], axis=0
    )
    nrm = np.maximum(np.linalg.norm(z, axis=1, keepdims=True), EPS)
    zn = (z / nrm).astype(np.float32)
    in_maps = []
    for c in range(NCORES):
        zr = np.roll(zn, -c * ROWS, axis=0)[:ZCOLS]
        zT = np.ascontiguousarray(zr.T).astype(ml_dtypes.bfloat16)
        in_maps.append({"zT": zT})
    return zn, in_maps


def combine(zn: np.ndarray, results) -> np.ndarray:
    S = np.zeros(N, dtype=np.float64)
    for c in range(NCORES):
        acc = np.asarray(results[c]["acc"], np.float64)   # [128, 24]
        cs = np.asarray(results[c]["cs"], np.float64)     # [4, 1536]
        for b in range(NBLK):
            rows = (c * ROWS + b * 128 + np.arange(128)) % N
            S[rows] += acc[:, 3 * b] + acc[:, 3 * b + 1] + acc[:, 3 * b + 2] - 1.0
        j = np.arange(128, 4992)
        seg = j // 512
        off = j % 512
        vals = np.empty(j.shape, np.float64)
        for s in range(10):
            m = seg == s
            if s < 4:
                vals[m] = cs[s, off[m]]                  # drain1: bank6 life1
            elif s < 8:
                vals[m] = cs[s - 4, 1024 + off[m]]       # final: bank7
            else:
                vals[m] = cs[s - 8, 512 + off[m]]        # final: bank6 life2
        rows = (c * ROWS + j) % N
        np.add.at(S, rows, vals)
    pos = (zn * np.roll(zn, -B, axis=0)).sum(axis=1)
    loss = np.log(S) + INV_T - INV_T * pos
    return np.asarray(np.float32(loss.mean()))


def kernel(emb0: np.ndarray, emb1: np.ndarray) -> np.ndarray:
    zn, in_maps = prepare_in_maps(emb0, emb1)
    res = bass_utils.run_bass_kernel_spmd(_get_nc(), in_maps, core_ids=list(range(NCORES)))
    return combine(zn, res.results)
